# revision 76
# baseline (speedup 1.0000x reference)
"""Trainium2 Bass kernel for nn_AttentionBlock (B=8, L=2048, E=512, FF=2048).

Data-parallel over batch: core b computes batch item b end-to-end. All
activations transposed ([feature, token], feature on partitions).

v2 over the 218us baseline:
  * fp16 residual datapath: xt/y/h/y2/out in fp16 -> DVE 2x_1p mode on the
    LN/residual elementwise chain, 1-pass stats matmuls (fp16 moving vs
    fp32r's ~360ns 2-pass), and half the xt/out HBM traffic.
  * Wide batched elementwise: [P, EO, LC] 3-D APs with 0-stride broadcasts
    do one DVE/ACT instruction per 2048 tokens instead of four.
  * LN1 (stats + apply) folded into the attention phase: each chunk's LN1
    pieces inject into the next chunk's sb loop, so the FFN phase's ACT/DVE
    load drops below the PE roofline and the psF1 relu ring deepens to 3
    banks (was 2) -> kills the ~345ns PE stalls + HAM re-throttle.
  * psAO/psF2 are single 4-bank PSUM tiles (one wide eviction op each).
  * LN2 stats/apply of chunk i inject into chunk i+1's ffn1 stream.
  * Tail: last chunk's final ffn2 accumulation step + epilogue run in
    column quarters so the post-matmul LN2 chain is ~128 wide.
"""
import math
from contextlib import ExitStack

import ml_dtypes
import numpy as np

import concourse.bass as bass
import concourse.bacc as bacc
import concourse.tile as tile
from concourse import mybir
from concourse.bass_utils import run_bass_kernel_spmd

P = 128
B, L, E, FF = 8, 2048, 512, 2048
NDOM = 32
EPS = 1e-5
SCALE = (1.0 / math.sqrt(E)) * 2.0 * math.log(NDOM)
EXPB = -5.0 * math.log(2.0)   # exp(s+EXPB)=exp(s)/32, cancels in softmax

EO = E // P           # 4  e-chunks
FO = FF // P          # 16 f-chunks
LC = 512              # l-chunk (matmul free dim)
NLC = L // LC         # 4  l-chunks
SB = L // P           # 16 s-blocks
NPR = EO // 2         # 2  DoubleRow eo-pairs

F32 = mybir.dt.float32
F16 = mybir.dt.float16
F8E4 = mybir.dt.float8e4
F8E5 = mybir.dt.float8e5
AF = mybir.ActivationFunctionType
OP = mybir.AluOpType
DR = mybir.MatmulPerfMode.DoubleRow

M_SC = 32.0           # m8 = 32*M (e4m3 normal range)
T_SC = 8.0            # t8 = 8*t  -> exp scale 1/8
WV_SC = 16.0          # wv8 = 16*Wv^T -> vt8 evict scale 1/16

_TRACE = False
LAST_RESULT = None
_CACHE = {}


def _build(ln1_trivial, ln2_trivial, b2_zero):
    nc = bacc.Bacc("TRN2", debug=False, target_bir_lowering=False, num_devices=B)

    xt_d = nc.dram_tensor("xt", [E, L], F16, kind="ExternalInput")
    x8_d = nc.dram_tensor("x8", [E, L], F8E4, kind="ExternalInput")
    m8_d = nc.dram_tensor("m8", [E, E], F8E4, kind="ExternalInput")
    wv8_d = nc.dram_tensor("wv8", [E, E], F8E4, kind="ExternalInput")
    w1t_d = nc.dram_tensor("w1t", [E, FF], F8E4, kind="ExternalInput")
    w2t_d = nc.dram_tensor("w2t", [FF, E], F8E4, kind="ExternalInput")
    b1_d = nc.dram_tensor("b1v", [FF], F32, kind="ExternalInput")
    b2_d = None if b2_zero else nc.dram_tensor("b2v", [E], F32, kind="ExternalInput")
    ln1w_d = ln1b_d = ln2w_d = ln2b_d = None
    if not ln1_trivial:
        ln1w_d = nc.dram_tensor("ln1w", [E], F32, kind="ExternalInput")
        ln1b_d = nc.dram_tensor("ln1b", [E], F32, kind="ExternalInput")
    if not ln2_trivial:
        ln2w_d = nc.dram_tensor("ln2w", [E], F32, kind="ExternalInput")
        ln2b_d = nc.dram_tensor("ln2b", [E], F32, kind="ExternalInput")
    out_d = nc.dram_tensor("outt", [E, L], F16, kind="ExternalOutput")

    xt_r = xt_d.ap().rearrange("(eo p) l -> p eo l", p=P)
    x8_r = x8_d.ap().rearrange("(eo p) l -> p eo l", p=P)
    m8_r = m8_d.ap().rearrange("(eo p) f -> p eo f", p=P)
    wv8_r = wv8_d.ap().rearrange("(eo p) f -> p eo f", p=P)
    w1t_r = w1t_d.ap().rearrange("(eo p) f -> p eo f", p=P)
    w2t_r = w2t_d.ap().rearrange("(fo p) e -> p fo e", p=P)
    out_r = out_d.ap().rearrange("(eo p) l -> p eo l", p=P)

    with tile.TileContext(nc) as tc, ExitStack() as stk:
        const = stk.enter_context(tc.tile_pool(name="const", bufs=1))
        px = stk.enter_context(tc.tile_pool(name="px", bufs=1))
        px8 = stk.enter_context(tc.tile_pool(name="px8", bufs=1))
        pstat = stk.enter_context(tc.tile_pool(name="pstat", bufs=1))
        pysq = stk.enter_context(tc.tile_pool(name="pysq", bufs=1))
        paon = stk.enter_context(tc.tile_pool(name="paon", bufs=1))
        ph = stk.enter_context(tc.tile_pool(name="ph", bufs=4))
        pw1 = stk.enter_context(tc.tile_pool(name="pw1", bufs=1))
        pw2 = stk.enter_context(tc.tile_pool(name="pw2", bufs=1))

        ones16 = const.tile([P, P], F16)
        ones_f2 = const.tile([P, 2, P], F32)
        ones8 = const.tile([P, 2, P], F8E4)
        eps_t = const.tile([P, 1], F32)
        expb_t = const.tile([P, 1], F32)
        b1_t = const.tile([P, FO], F32)
        nc.vector.memset(ones16[:], 1.0)
        nc.vector.memset(ones_f2[:], 1.0)
        nc.vector.tensor_copy(ones8[:], ones_f2[:])
        nc.vector.memset(eps_t[:], EPS)
        nc.vector.memset(expb_t[:], EXPB)
        warm = const.tile([P, 8], F32)
        nc.vector.memset(warm[:], 0.0)
        # touch Exp so the ACT table load hides under DMA-queue startup
        nc.scalar.activation(warm[:], warm[:], AF.Exp, bias=expb_t[:])
        b1_r = b1_d.ap().rearrange("(fo p) -> p fo", p=P)
        nc.scalar.dma_start(b1_t[:], b1_r)
        b2_t = None
        if b2_d is not None:
            b2_t = const.tile([P, EO], F32)
            nc.scalar.dma_start(b2_t[:], b2_d.ap().rearrange("(eo p) -> p eo", p=P))
        ln1w_t = ln1b_t = ln2w_t = ln2b_t = None
        if ln1w_d is not None:
            ln1w_t = const.tile([P, EO], F32)
            ln1b_t = const.tile([P, EO], F32)
            nc.scalar.dma_start(ln1w_t[:], ln1w_d.ap().rearrange("(eo p) -> p eo", p=P))
            nc.scalar.dma_start(ln1b_t[:], ln1b_d.ap().rearrange("(eo p) -> p eo", p=P))
        if ln2w_d is not None:
            ln2w_t = const.tile([P, EO], F32)
            ln2b_t = const.tile([P, EO], F32)
            nc.scalar.dma_start(ln2w_t[:], ln2w_d.ap().rearrange("(eo p) -> p eo", p=P))
            nc.scalar.dma_start(ln2b_t[:], ln2b_d.ap().rearrange("(eo p) -> p eo", p=P))

        xt = px.tile([P, EO, L], F16)           # x^T, becomes y = x + attn
        x8 = px8.tile([P, EO, L], F8E4)         # e4m3 copy for DR matmuls
        w1t = pw1.tile([P, EO, FF], F8E4)
        w2t = pw2.tile([P, FO, E], F8E4)
        state = {}

        def ln1_stats_a(lc, psSB, tag="sums"):
            """s1 partition sums of y(lc)."""
            ls = lc * LC
            s_ps = psSB.tile([P, LC], F32, tag=tag, name=f"s1_{lc}")
            for ec in range(EO):
                nc.tensor.matmul(s_ps[:], ones16[:], xt[:, ec, ls:ls + LC],
                                 start=(ec == 0), stop=(ec == EO - 1))
            state[("s1", lc)] = s_ps

        def ln1_evict(lc):
            s_ps = state.pop(("s1", lc))
            nm16 = pstat.tile([P, LC], F16, tag="nm1", name=f"nm1_{lc}",
                              bufs=3)
            nc.scalar.activation(nm16[:], s_ps[:], AF.Copy, scale=-1.0 / E)
            state[("nm1", lc)] = nm16

        def ln1_stats_b(lc, pool, tag="sums"):
            """s2 partition sums of ysq(lc)."""
            ysq = state[("ysq1", lc)]
            s2_ps = pool.tile([P, LC], F32, tag=tag, name=f"s2_{lc}")
            for ec in range(EO):
                nc.tensor.matmul(s2_ps[:], ones16[:], ysq[:, ec, :],
                                 start=(ec == 0), stop=(ec == EO - 1))
            state[("s2", lc)] = s2_ps

        def ln1_rest_a(lc):
            """variance into SBUF (DVE only — attention-phase safe)."""
            nm16 = state[("nm1", lc)]
            s2_ps = state.pop(("s2", lc))
            state.pop(("ysq1", lc))
            msqh = pstat.tile([P, LC], F16, tag="msqh", name=f"msqh1_{lc}")
            ex2 = pstat.tile([P, LC], F32, tag="ex2", name=f"ex21_{lc}",
                             bufs=3)
            nc.vector.tensor_tensor(msqh[:], nm16[:], nm16[:], OP.mult)
            nc.vector.scalar_tensor_tensor(ex2[:], s2_ps[:], 1.0 / E,
                                           msqh[:], OP.mult, OP.subtract)
            state[("ex2", lc)] = ex2

        def ln1_rest_b(lc):
            """rstd from variance; the ACT Sqrt lives here so it can be
            scheduled away from the softmax Exp stream (table-set swap)."""
            ex2 = state.pop(("ex2", lc))
            rstd = pstat.tile([P, LC], F32, tag="rstd1", name=f"rstd1_{lc}")
            rstd16 = pstat.tile([P, LC], F16, tag="rstd1h", name=f"rstd1h_{lc}")
            nc.scalar.activation(ex2[:], ex2[:], AF.Sqrt, bias=eps_t[:])
            nc.vector.reciprocal_approx_fast(rstd[:], ex2[:])
            nc.vector.tensor_copy(rstd16[:], rstd[:])
            state[("rstd1", lc)] = rstd16

        def ln1_apply(lc):
            nm16 = state.pop(("nm1", lc))
            rstd16 = state.pop(("rstd1", lc))
            ls = lc * LC
            y3 = xt[:, :, ls:ls + LC]
            t = pstat.tile([P, EO, LC], F16, tag="t1", name=f"t1_{lc}")
            h = ph.tile([P, EO, LC], F16, tag="h", name=f"h{lc}")
            h8 = ph.tile([P, EO, LC], F8E4, tag="h8", name=f"h8_{lc}")
            nc.vector.tensor_tensor(
                t[:], y3, nm16[:, None, :].to_broadcast((P, EO, LC)), OP.add)
            nc.vector.tensor_tensor(
                h[:], t[:], rstd16[:, None, :].to_broadcast((P, EO, LC)),
                OP.mult)
            if not ln1_trivial:
                for ec in range(EO):
                    nc.scalar.activation(h[:, ec, :], h[:, ec, :], AF.Identity,
                                         bias=ln1b_t[:, ec:ec + 1],
                                         scale=ln1w_t[:, ec:ec + 1])
            nc.vector.tensor_copy(h8[:], h[:])
            state[("h", lc)] = (h, h8)

        # ---------------- phase A+B: loads + attention ----------------
        with tc.tile_pool(name="pkv", bufs=1) as pkv, \
             tc.tile_pool(name="pm8", bufs=1) as pm8, \
             tc.tile_pool(name="pwv", bufs=1) as pwv, \
             tc.tile_pool(name="psMM", bufs=3, space="PSUM") as psMM:
            m8 = pm8.tile([P, EO, E], F8E4)     # 32*M
            vt8 = pkv.tile([P, SB, E], F8E4)    # v natural [s, e]
            wv8 = pwv.tile([P, EO, E], F8E4)

            # PE warm-up: const matmuls bridge the DMA-startup head gap and
            # ramp HAM to K=8/8 before real work
            warm_mv = pwv.tile([P, 256], F16)
            nc.vector.memset(warm_mv[:], 1.0)
            warm_ps = psMM.tile([P, 256], F32, tag="mm", name="warmps")
            warm_sb = pwv.tile([P, 8], F32)
            for k in range(16):
                nc.tensor.matmul(warm_ps[:], ones16[:], warm_mv[:],
                                 start=(k == 0), stop=(k == 15))
            nc.scalar.activation(warm_sb[:], warm_ps[:, 0:8], AF.Copy)

            # DMA plan: x8/m8 on sync+gpsimd (first-need), wv8 on the scalar
            # ring, xt on the vector ring — four rings pull the critical
            # chunk-0 operands in parallel.
            nc.sync.dma_start(m8[:, 0:2, :], m8_r[:, 0:2, :])
            nc.gpsimd.dma_start(m8[:, 2:4, :], m8_r[:, 2:4, :])
            nc.sync.dma_start(x8[:, 0:2, 0:LC], x8_r[:, 0:2, 0:LC])
            nc.gpsimd.dma_start(x8[:, 2:4, 0:LC], x8_r[:, 2:4, 0:LC])
            nc.gpsimd.dma_start(wv8[:, 0:2, :], wv8_r[:, 0:2, :])
            nc.sync.dma_start(wv8[:, 2:4, :], wv8_r[:, 2:4, :])
            nc.sync.dma_start(x8[:, 0:2, LC:], x8_r[:, 0:2, LC:])
            nc.gpsimd.dma_start(x8[:, 2:4, LC:], x8_r[:, 2:4, LC:])
            for eo in range(EO):
                (nc.sync, nc.gpsimd)[eo % 2].dma_start(
                    xt[:, eo, :], xt_r[:, eo, :])
            nc.sync.dma_start(w1t[:, 0:2, :], w1t_r[:, 0:2, :])
            nc.gpsimd.dma_start(w1t[:, 2:4, :], w1t_r[:, 2:4, :])
            nc.sync.dma_start(w2t[:, 0:8, :], w2t_r[:, 0:8, :])
            nc.gpsimd.dma_start(w2t[:, 8:16, :], w2t_r[:, 8:16, :])

            with (
                tc.tile_pool(name="pq", bufs=2) as pq,
                tc.tile_pool(name="pp", bufs=2) as pp,
                tc.tile_pool(name="psAO", bufs=1, space="PSUM") as psAO,
                tc.tile_pool(name="psCS", bufs=1, space="PSUM") as psCS,
            ):
                def t_proj_part(t8, lc, eb, on_act=None):
                    ls = lc * LC
                    tp = psMM.tile([P, LC], F32, tag="mm",
                                   name=f"tp{lc}_{eb}")
                    for pr in range(NPR):
                        nc.tensor.matmul(
                            tp[:], m8[:, 2 * pr:2 * pr + 2,
                                      eb * P:(eb + 1) * P],
                            x8[:, 2 * pr:2 * pr + 2, ls:ls + LC],
                            start=(pr == 0), stop=(pr == NPR - 1),
                            perf_mode=DR)
                    if on_act is None:
                        on_act = eb % 2 == 1
                    if on_act:
                        nc.scalar.activation(t8[:, eb, :], tp[:], AF.Copy,
                                             scale=T_SC / M_SC)
                    else:
                        nc.vector.tensor_scalar_mul(t8[:, eb, :], tp[:],
                                                    T_SC / M_SC)

                def t_proj(lc):
                    t8 = pq.tile([P, EO, LC], F8E4, tag="q", name=f"t8_{lc}")
                    for eb in range(EO):
                        t_proj_part(t8, lc, eb)
                    return t8

                def v_proj(sb):
                    vp = psMM.tile([P, E], F32, tag="mm", name=f"vp{sb}")
                    for pr in range(NPR):
                        nc.tensor.matmul(
                            vp[:], x8[:, 2 * pr:2 * pr + 2,
                                      sb * P:(sb + 1) * P],
                            wv8[:, 2 * pr:2 * pr + 2, :],
                            start=(pr == 0), stop=(pr == NPR - 1),
                            perf_mode=DR)
                    if sb % 2 == 0:
                        nc.scalar.activation(vt8[:, sb, :], vp[:], AF.Copy,
                                             scale=1.0 / WV_SC)
                    else:
                        nc.vector.tensor_scalar_mul(vt8[:, sb, :], vp[:],
                                                    1.0 / WV_SC)

                t8s = {0: t_proj(0)}
                ctxs = {}

                def make_ctx(lc):
                    t8 = t8s.pop(lc)
                    pexp = pp.tile([P, SB, LC], F8E5, tag="pexp",
                                   name=f"pexp{lc}")
                    st_ps = []

                    def scores(sb, t8=t8, st_ps=st_ps, lc=lc):
                        sp = psMM.tile([P, LC], F32, tag="mm",
                                       name=f"sp{lc}_{sb}")
                        for pr in range(NPR):
                            nc.tensor.matmul(
                                sp[:], x8[:, 2 * pr:2 * pr + 2,
                                          sb * P:(sb + 1) * P],
                                t8[:, 2 * pr:2 * pr + 2, :],
                                start=(pr == 0), stop=(pr == NPR - 1),
                                perf_mode=DR)
                        st_ps.append(sp)

                    def expevict(sb, pexp=pexp, st_ps=st_ps):
                        nc.scalar.activation(pexp[:, sb, :], st_ps[sb][:],
                                             AF.Exp, bias=expb_t[:],
                                             scale=1.0 / T_SC)

                    return {"pexp": pexp, "scores": scores, "exp": expevict,
                            "next_sb": 0}

                for lc in range(NLC):
                    ls = lc * LC
                    ctx = ctxs.pop(lc, None)
                    if ctx is None:
                        ctx = make_ctx(lc)
                    pexp = ctx["pexp"]
                    ao = psAO.tile([P, EO, LC], F32, tag="ao",
                                   name=f"ao{lc}")
                    cs = psCS.tile([P, LC], F32, tag="cs", name=f"cs{lc}")

                    # previous chunk's LN1 pieces inject into this chunk
                    inject = {}
                    if lc >= 1:
                        pl = lc - 1
                        inject = {
                            5: lambda pl=pl: ln1_stats_a(pl, psMM, tag="mm"),
                            7: lambda pl=pl: ln1_evict(pl),
                            9: lambda pl=pl: ln1_stats_b(pl, psMM, tag="mm"),
                            11: lambda pl=pl: ln1_rest_a(pl),
                        }
                        if pl == 0:
                            # chunk 0's h8 is needed at FFN(0) start, so its
                            # sqrt+apply can't move into the FFN stream;
                            # sb15 puts the sqrt's table swap AFTER the
                            # pre-run exps of chunk 2
                            def _fin0():
                                ln1_rest_b(0)
                                ln1_apply(0)
                            inject[15] = _fin0

                    def av_pair(j, pexp=pexp, ao=ao, cs=cs):
                        nc.tensor.matmul(cs[:], ones8[:],
                                         pexp[:, 2 * j:2 * j + 2, :],
                                         start=(j == 0), stop=(j == SB // 2 - 1),
                                         perf_mode=DR)
                        for eb in range(EO):
                            nc.tensor.matmul(
                                ao[:, eb, :],
                                vt8[:, 2 * j:2 * j + 2, eb * P:(eb + 1) * P],
                                pexp[:, 2 * j:2 * j + 2, :],
                                start=(j == 0), stop=(j == SB // 2 - 1),
                                perf_mode=DR)

                    for sb in range(SB):
                        if lc == 0:
                            v_proj(sb)
                        if sb >= ctx["next_sb"]:
                            ctx["scores"](sb)
                            ctx["exp"](sb)
                            ctx["next_sb"] = sb + 1
                        if sb in (5, 6, 7, 9, 11, 13, 15):
                            av_pair({5: 0, 6: 1, 7: 2, 9: 3, 11: 4,
                                     13: 5, 15: 6}[sb])
                        if sb in inject:
                            inject[sb]()
                        if lc == 0 and sb in (8, 9):
                            # chunk 1's t8 early enough that its first two
                            # score blocks can pre-run at sb13/14
                            if sb == 8:
                                t8n = pq.tile([P, EO, LC], F8E4, tag="q",
                                              name=f"t8_{lc + 1}")
                                t8s[lc + 1] = t8n
                            for eb in (0, 1) if sb == 8 else (2, 3):
                                t_proj_part(t8s[lc + 1], lc + 1, eb)
                        if lc >= 1 and lc + 1 < NLC and sb in (0, 1):
                            # chunk lc+1's t8 computed at the TOP of chunk lc:
                            # fills the PE while the previous chunk's DVE
                            # epilogue (rcs/aon/y) drains, so av_pair(0) and
                            # the s1 stats don't stall at the boundary.
                            if sb == 0:
                                t8n = pq.tile([P, EO, LC], F8E4, tag="q",
                                              name=f"t8_{lc + 1}")
                                t8s[lc + 1] = t8n
                            for eb in (0, 1) if sb == 0 else (2, 3):
                                t_proj_part(t8s[lc + 1], lc + 1, eb,
                                            on_act=True)
                        if lc + 1 < NLC and sb in (13, 14):
                            # pre-run the next chunk's first two score blocks
                            # + exps: av_pair(0)'s inputs are then ready even
                            # if a table-swap lands at the chunk boundary
                            if sb == 13:
                                ctxs[lc + 1] = make_ctx(lc + 1)
                            nctx = ctxs[lc + 1]
                            nctx["scores"](nctx["next_sb"])
                            nctx["exp"](nctx["next_sb"])
                            nctx["next_sb"] += 1
                    av_pair(SB // 2 - 1)

                    # epilogue: y = x + ao*rcs (in place, fp16) + ysq.
                    # chunk 3 evicts ao RAW (split across ACT+DVE) and defers
                    # the normalize/y/ysq into phase C, so the phase-boundary
                    # pool barrier only waits ~2.5us of evictions instead of
                    # the full serial DVE chain.
                    rcs = pstat.tile([P, LC], F32, tag="rcs",
                                     name=f"rcs{lc}")
                    nc.vector.reciprocal_approx_fast(rcs[:], cs[:])
                    aon = paon.tile([P, EO, LC], F16, tag="aon",
                                    name=f"aon{lc}")
                    if lc < NLC - 1:
                        nc.vector.tensor_tensor(
                            aon[:], ao[:],
                            rcs[:, None, :].to_broadcast((P, EO, LC)),
                            OP.mult)
                        nc.vector.tensor_tensor(
                            xt[:, :, ls:ls + LC], xt[:, :, ls:ls + LC],
                            aon[:], OP.add)
                        ysq = pysq.tile([P, EO, LC], F16, tag="ysq",
                                        name=f"ysq1_{lc}")
                        nc.vector.tensor_tensor(ysq[:], xt[:, :, ls:ls + LC],
                                                xt[:, :, ls:ls + LC], OP.mult)
                        state[("ysq1", lc)] = ysq
                    else:
                        nc.scalar.activation(aon[:, 0:2, :], ao[:, 0:2, :],
                                             AF.Copy)
                        nc.vector.tensor_copy(aon[:, 2:4, :], ao[:, 2:4, :])
                        state["aon3"] = (aon, rcs)

        # ---------------- phase C: FFN + LN2 per l-chunk ----------------
        with (
            tc.tile_pool(name="py2", bufs=2) as py2,
            tc.tile_pool(name="pysq2", bufs=2) as pysq2,
            tc.tile_pool(name="prelu", bufs=1) as prelu,
            tc.tile_pool(name="pout", bufs=2) as pout,
            tc.tile_pool(name="psF1", bufs=3, space="PSUM") as psF1,
            tc.tile_pool(name="psF2", bufs=1, space="PSUM") as psF2,
            tc.tile_pool(name="psS", bufs=1, space="PSUM") as psS,
        ):
            def ffn1(i, fo):
                relu1 = state[("relu", i)]
                _, h8 = state[("h", i)]
                fp = psF1.tile([P, LC], F32, tag="f1", name=f"fp{i}_{fo}")
                w1s = w1t[:, :, fo * P:(fo + 1) * P]
                for pr in range(NPR):
                    nc.tensor.matmul(fp[:], w1s[:, 2 * pr:2 * pr + 2, :],
                                     h8[:, 2 * pr:2 * pr + 2, :],
                                     start=(pr == 0), stop=(pr == NPR - 1),
                                     perf_mode=DR)
                nc.scalar.activation(relu1[:, fo, :], fp[:], AF.Relu,
                                     bias=b1_t[:, fo:fo + 1],
                                     scale=1.0 / M_SC)

            def ffn2(i, j, last_group_cols=None):
                relu1 = state[("relu", i)]
                ao2 = state[("ao2", i)]
                if last_group_cols is None:
                    for eb in range(EO):
                        nc.tensor.matmul(
                            ao2[:, eb, :],
                            w2t[:, 2 * j:2 * j + 2, eb * P:(eb + 1) * P],
                            relu1[:, 2 * j:2 * j + 2, :],
                            start=(j == 0), stop=(j == FO // 2 - 1),
                            perf_mode=DR)
                else:
                    cl, chi = last_group_cols
                    for eb in range(EO):
                        nc.tensor.matmul(
                            ao2[:, eb, cl:chi],
                            w2t[:, 2 * j:2 * j + 2, eb * P:(eb + 1) * P],
                            relu1[:, 2 * j:2 * j + 2, cl:chi],
                            start=False, stop=True,
                            perf_mode=DR, skip_group_check=True)

            def resid2(i, cols=None):
                """y2 = ffn*sc + h (+b2) in fp16, plus ysq2."""
                cl, chi = (0, LC) if cols is None else cols
                w = chi - cl
                ao2 = state[("ao2", i)]
                h, _ = state[("h", i)]
                if cols is None or cl == 0:
                    y2 = py2.tile([P, EO, LC], F16, tag="y2", name=f"y2_{i}")
                    ysq2 = pysq2.tile([P, EO, LC], F16, tag="ysq2",
                                      name=f"ysq2_{i}")
                    state[("y2", i)] = (y2, ysq2)
                y2, ysq2 = state[("y2", i)]
                if cols is not None:
                    # tail quarters: ACT evicts ao2 raw so the DVE stt runs
                    # all-SBUF fp16 at 2x (DVE is the tail's critical path)
                    raw = pstat.tile([P, EO, w], F16, tag="ffraw",
                                     name=f"ffraw_{cl}")
                    nc.scalar.activation(raw[:], ao2[:, :, cl:chi], AF.Copy,
                                         scale=1.0 / M_SC)
                    nc.vector.tensor_tensor(y2[:, :, cl:chi], raw[:],
                                            h[:, :, cl:chi], OP.add)
                else:
                    nc.vector.scalar_tensor_tensor(
                        y2[:, :, cl:chi], ao2[:, :, cl:chi], 1.0 / M_SC,
                        h[:, :, cl:chi], OP.mult, OP.add)
                if b2_t is not None:
                    for ec in range(EO):
                        nc.vector.tensor_tensor(
                            y2[:, ec, cl:chi], y2[:, ec, cl:chi],
                            b2_t[:, ec:ec + 1].to_broadcast((P, w)), OP.add)
                nc.vector.tensor_tensor(ysq2[:, :, cl:chi], y2[:, :, cl:chi],
                                        y2[:, :, cl:chi], OP.mult)

            def ln2_stats_a(i):
                y2, _ = state[("y2", i)]
                s_ps = psS.tile([P, LC], F32, tag="sums", name=f"s3_{i}")
                for ec in range(EO):
                    nc.tensor.matmul(s_ps[:], ones16[:], y2[:, ec, :],
                                     start=(ec == 0), stop=(ec == EO - 1))
                state[("s3", i)] = s_ps

            def ln2_evict(i):
                s_ps = state.pop(("s3", i))
                nm16 = pstat.tile([P, LC], F16, tag="nm2", name=f"nm2_{i}")
                nc.scalar.activation(nm16[:], s_ps[:], AF.Copy, scale=-1.0 / E)
                state[("nm2", i)] = nm16

            def ln2_stats_b(i):
                _, ysq2 = state[("y2", i)]
                s2_ps = psS.tile([P, LC], F32, tag="sums", name=f"s4_{i}")
                for ec in range(EO):
                    nc.tensor.matmul(s2_ps[:], ones16[:], ysq2[:, ec, :],
                                     start=(ec == 0), stop=(ec == EO - 1))
                state[("s4", i)] = s2_ps

            def ln2_rest(i):
                nm16 = state[("nm2", i)]
                s2_ps = state.pop(("s4", i))
                msqh = pstat.tile([P, LC], F16, tag="msqh2", name=f"msqh2_{i}")
                ex2 = pstat.tile([P, LC], F32, tag="ex2b", name=f"ex22_{i}")
                rstd = pstat.tile([P, LC], F32, tag="rstd2", name=f"rstd2_{i}")
                rstd16 = pstat.tile([P, LC], F16, tag="rstd2h",
                                    name=f"rstd2h_{i}")
                nc.vector.tensor_tensor(msqh[:], nm16[:], nm16[:], OP.mult)
                nc.vector.scalar_tensor_tensor(ex2[:], s2_ps[:], 1.0 / E,
                                               msqh[:], OP.mult, OP.subtract)
                nc.scalar.activation(ex2[:], ex2[:], AF.Sqrt, bias=eps_t[:])
                nc.vector.reciprocal_approx_fast(rstd[:], ex2[:])
                nc.vector.tensor_copy(rstd16[:], rstd[:])
                state[("rstd2", i)] = rstd16

            def ln2_apply(i):
                nm16 = state.pop(("nm2", i))
                rstd16 = state.pop(("rstd2", i))
                y2, _ = state.pop(("y2", i))
                t2 = pstat.tile([P, EO, LC], F16, tag="t2", name=f"t2_{i}")
                outt = pout.tile([P, EO, LC], F16, tag="out", name=f"out{i}")
                nc.vector.tensor_tensor(
                    t2[:], y2[:], nm16[:, None, :].to_broadcast((P, EO, LC)),
                    OP.add)
                nc.vector.tensor_tensor(
                    outt[:], t2[:],
                    rstd16[:, None, :].to_broadcast((P, EO, LC)), OP.mult)
                if not ln2_trivial:
                    for ec in range(EO):
                        nc.scalar.activation(outt[:, ec, :], outt[:, ec, :],
                                             AF.Identity,
                                             bias=ln2b_t[:, ec:ec + 1],
                                             scale=ln2w_t[:, ec:ec + 1])
                state[("out", i)] = outt

            def out_dma(i):
                outt = state.pop(("out", i))
                ls = i * LC
                nc.sync.dma_start(out_r[:, :, ls:ls + LC], outt[:])

            def c3_finish_y():
                # chunk 3's deferred normalize + residual (reads SBUF only)
                aon, rcs = state.pop("aon3")
                ls3 = 3 * LC
                rcs16 = pstat.tile([P, LC], F16, tag="rcs16", name="rcs16_3")
                nc.vector.tensor_copy(rcs16[:], rcs[:])
                nc.vector.tensor_tensor(
                    aon[:], aon[:],
                    rcs16[:, None, :].to_broadcast((P, EO, LC)), OP.mult)
                nc.vector.tensor_tensor(xt[:, :, ls3:], xt[:, :, ls3:],
                                        aon[:], OP.add)

            def c3_stats_a():
                ln1_stats_a(3, psS)
                ln1_evict(3)

            def c3_stats_b():
                ls3 = 3 * LC
                ysq3 = pysq.tile([P, EO, LC], F16, tag="ysq", name="ysq1_3")
                nc.vector.tensor_tensor(ysq3[:], xt[:, :, ls3:],
                                        xt[:, :, ls3:], OP.mult)
                state[("ysq1", 3)] = ysq3
                ln1_stats_b(3, psS)

            def c3_rest():
                ln1_rest_a(3)
                # LN1(1)'s sqrt+apply: the table swap merges with the
                # natural attention->FFN set transition
                ln1_rest_b(1)
                ln1_apply(1)

            for i in range(NLC):
                last = i == NLC - 1
                relu1 = prelu.tile([P, FO, LC], F8E4, tag="relu1",
                                   name=f"relu1_{i}")
                ao2 = psF2.tile([P, EO, LC], F32, tag="ao2", name=f"ao2_{i}")
                state[("relu", i)] = relu1
                state[("ao2", i)] = ao2

                # previous chunk's LN2 pieces + chunk i+1's LN1 finish
                # (sqrt sits in the relu/sqrt table set: no swaps here)
                if i == 0:
                    inject = {1: c3_finish_y, 5: c3_stats_a,
                              7: c3_stats_b, 8: c3_rest}
                else:
                    pi = i - 1
                    inject = {
                        4: lambda pi=pi: ln2_stats_a(pi),
                        5: lambda pi=pi: ln2_evict(pi),
                        6: lambda pi=pi: ln2_stats_b(pi),
                        7: lambda pi=pi: ln2_rest(pi),
                        8: lambda pi=pi: ln2_apply(pi),
                        9: lambda pi=pi: out_dma(pi),
                    }
                if i + 2 < NLC:
                    nl = i + 2
                    inject[12] = lambda nl=nl: ln1_rest_b(nl)
                    inject[13] = lambda nl=nl: ln1_apply(nl)

                for fo in range(FO):
                    ffn1(i, fo)
                    if fo in inject:
                        inject[fo]()
                    if last:
                        if fo >= 3 and fo % 2 == 1:
                            ffn2(i, (fo - 3) // 2)
                    elif fo >= 4 and fo % 2 == 0:
                        ffn2(i, (fo - 4) // 2)
                if not last:
                    ffn2(i, FO // 2 - 2)
                    ffn2(i, FO // 2 - 1)
                    resid2(i)
                else:
                    # tail: final accumulation step in column quarters so the
                    # LN2 chain of each quarter overlaps the next's matmuls
                    QW = LC // 4

                    def tail_stats(q):
                        ql, qh = q * QW, (q + 1) * QW
                        resid2(i, (ql, qh))
                        st = psS.tile([P, 2, QW], F32, tag="sums",
                                      name=f"stail_{q}")
                        y2, ysq2 = state[("y2", i)]
                        for ec in range(EO):
                            nc.tensor.matmul(st[:, 0, :], ones16[:],
                                             y2[:, ec, ql:qh],
                                             start=(ec == 0), stop=(ec == EO - 1))
                        for ec in range(EO):
                            nc.tensor.matmul(st[:, 1, :], ones16[:],
                                             ysq2[:, ec, ql:qh],
                                             start=(ec == 0), stop=(ec == EO - 1))
                        state[("stail", q)] = st

                    def tail_ln(q):
                        # latency-lean variant: ACT Square (free in the
                        # relu/sqrt table set) instead of a DVE msqh hop,
                        # fp32 rstd so no fp16 cast hop
                        ql, qh = q * QW, (q + 1) * QW
                        st = state.pop(("stail", q))
                        y2, _ = state[("y2", i)]
                        nm16 = pstat.tile([P, QW], F16, tag="nmt",
                                          name=f"nmt_{q}")
                        msq = pstat.tile([P, QW], F32, tag="msqt",
                                         name=f"msqt_{q}")
                        ex2 = pstat.tile([P, QW], F32, tag="ex2t",
                                         name=f"ex2t_{q}")
                        rstdf = pstat.tile([P, QW], F32, tag="rstdtf",
                                           name=f"rstdtf_{q}")
                        nc.scalar.activation(nm16[:], st[:, 0, :], AF.Copy,
                                             scale=-1.0 / E)
                        nc.scalar.activation(msq[:], st[:, 0, :], AF.Square,
                                             scale=1.0 / E)
                        nc.vector.scalar_tensor_tensor(ex2[:], st[:, 1, :],
                                                       1.0 / E, msq[:],
                                                       OP.mult, OP.subtract)
                        nc.scalar.activation(ex2[:], ex2[:], AF.Sqrt,
                                             bias=eps_t[:])
                        nc.vector.reciprocal_approx_fast(rstdf[:], ex2[:])
                        t2 = pstat.tile([P, EO, QW], F16, tag="t2t",
                                        name=f"t2t_{q}")
                        outt = pout.tile([P, EO, QW], F16, tag="outt",
                                         name=f"outq_{q}", bufs=4)
                        nc.vector.tensor_tensor(
                            t2[:], y2[:, :, ql:qh],
                            nm16[:, None, :].to_broadcast((P, EO, QW)),
                            OP.add)
                        nc.vector.tensor_tensor(
                            outt[:], t2[:],
                            rstdf[:, None, :].to_broadcast((P, EO, QW)),
                            OP.mult)
                        if not ln2_trivial:
                            for ec in range(EO):
                                nc.scalar.activation(
                                    outt[:, ec, :], outt[:, ec, :],
                                    AF.Identity, bias=ln2b_t[:, ec:ec + 1],
                                    scale=ln2w_t[:, ec:ec + 1])
                        ls = i * LC + ql
                        (nc.gpsimd, nc.scalar, nc.sync, nc.sync)[q].dma_start(
                            out_r[:, :, ls:ls + QW], outt[:])

                    ffn2(i, FO // 2 - 1, (0, QW))
                    ffn2(i, FO // 2 - 1, (QW, 2 * QW))
                    tail_stats(0)
                    ffn2(i, FO // 2 - 1, (2 * QW, 3 * QW))
                    tail_stats(1)
                    ffn2(i, FO // 2 - 1, (3 * QW, LC))
                    tail_ln(0)
                    tail_stats(2)
                    tail_ln(1)
                    tail_stats(3)
                    tail_ln(2)
                    tail_ln(3)
                    state.pop(("y2", i))

    nc.compile()
    return nc


def kernel(x, in_proj_w, ln1_w, ln1_b, ln2_w, ln2_b, w1, b1, w2, b2):
    global LAST_RESULT
    x = np.asarray(x, dtype=np.float32)
    in_proj_w = np.asarray(in_proj_w, dtype=np.float32)
    w1 = np.asarray(w1, dtype=np.float32)
    w2 = np.asarray(w2, dtype=np.float32)
    b1 = np.asarray(b1, dtype=np.float32)
    b2 = np.asarray(b2, dtype=np.float32)
    ln1_w = np.asarray(ln1_w, dtype=np.float32)
    ln1_b = np.asarray(ln1_b, dtype=np.float32)
    ln2_w = np.asarray(ln2_w, dtype=np.float32)
    ln2_b = np.asarray(ln2_b, dtype=np.float32)

    ln1_trivial = bool(np.all(ln1_w == 1.0) and np.all(ln1_b == 0.0))
    ln2_trivial = bool(np.all(ln2_w == 1.0) and np.all(ln2_b == 0.0))
    b2_zero = bool(np.all(b2 == 0.0))

    key = (ln1_trivial, ln2_trivial, b2_zero)
    if key not in _CACHE:
        _CACHE[key] = _build(*key)
    nc = _CACHE[key]

    E4NP = ml_dtypes.float8_e4m3
    wq = in_proj_w[:E].astype(np.float64)
    wk = in_proj_w[E:2 * E].astype(np.float64)
    m8 = (M_SC * SCALE * (wq.T @ wk)).astype(np.float32).astype(E4NP)  # [E, E]
    wv8 = (WV_SC * in_proj_w[2 * E:].T).astype(E4NP)
    w1t = (M_SC * w1.T).astype(E4NP)                             # [E, FF]
    w2t = (M_SC * w2.T).astype(E4NP)                             # [FF, E]

    in_maps = []
    for bb in range(B):
        xtb = np.ascontiguousarray(x[bb].T)
        m = {
            "xt": xtb.astype(np.float16),                        # [E, L]
            "x8": xtb.astype(E4NP),
            "m8": m8, "wv8": wv8,
            "w1t": w1t, "w2t": w2t, "b1v": b1,
        }
        if not b2_zero:
            m["b2v"] = b2
        if not ln1_trivial:
            m["ln1w"] = ln1_w
            m["ln1b"] = ln1_b
        if not ln2_trivial:
            m["ln2w"] = ln2_w
            m["ln2b"] = ln2_b
        in_maps.append(m)

    res = run_bass_kernel_spmd(nc, in_maps, list(range(B)), trace=_TRACE)
    LAST_RESULT = res
    out = np.stack([np.ascontiguousarray(res.results[bb]["outt"].T)
                    for bb in range(B)])
    return out.astype(np.float32)
